# revision 102
# baseline (speedup 1.0000x reference)
"""Trainium2 Bass kernel for nn_Attention (B=4, N=2048, C=768, H=12, D=64).

Sharding: core c -> batch b=c//2, head-group hg=c%2 (6 heads each).
qkv_w column-parallel, proj_w row-parallel (host sums the 2 partials per b).

v2 design notes (vs the fp32r baseline):
  - All weight/activation matmul operands are bf16 (qkv, scores, proj);
    exp outputs (es) and V (vA) stay fp32r so the AV matmuls are pure fp32r.
    Matmul cost is 1.0 cycles/row either way; bf16 buys SBUF room, DVE
    2x/4x modes, and XBAR DMA transposes (which need a 2-byte dtype).
  - q/k feature-major transposes are hybrid: the latency-critical pair-0
    head uses PE transposes (bf16 identity, staged in the qkp psum tail);
    pairs 1-2 prep has a whole pair of slack, so the XBAR DMA engines
    transpose straight into qkT, off the PE and with no psum or copy.
  - The Activation engine runs only the softmax exp plus a few head-time
    copies; all other psum->sbuf traffic is on DVE (gpsimd cannot access
    PSUM on real hardware, although the cost model allows it).
  - vA carries a 64-wide ones block so each AV matmul emits the softmax
    denominator already broadcast across partitions (matmul cost depends
    only on the moving free size, so the wider stationary is free); the
    normalize is then just reciprocal + multiply on DVE.
  - Prep (for the next head-pair) and projection are emitted in small PE
    quanta between attention kpairs, rate-paced so the PE stream has no
    long stalls; transposes are deferred ~1-2 tiles behind their chain so
    the PE never waits on the DVE rope/norm chain.
  - Inputs arrive as few merged DMAs (the HWDGE dispatcher serializes at
    625ns/DMA while transfers run at full aggregate bandwidth), tables are
    host-pre-shuffled into the on-chip layout for a full-rate contiguous
    transfer, and junk matmuls at t~0 keep the PE p-state ramp warm.
"""
import sys

sys.path.insert(0, "/opt/trn_rl_repo")

import ml_dtypes
import numpy as np
import concourse.bass as bass
import concourse.mybir as mybir
import concourse.tile as tile
from concourse import bacc
from concourse.bass_utils import run_bass_kernel_spmd
from concourse.masks import make_identity

dt = mybir.dt
AF = mybir.ActivationFunctionType
ALU = mybir.AluOpType
AX = mybir.AxisListType

B, N, C = 4, 2048, 768
H, D = 12, 64
HPC = 6            # heads per core
EPS = 1e-6
NT = N // 128      # 16 token tiles
NCHUNK = C // 128  # 6 contraction chunks
SCALE = D ** -0.5  # 0.125
NG = 4             # qi groups
G = N // NG        # 512 per group

# Schraudolph exp constants (round-to-nearest fp32->int32 convert):
# exp(x) ~= bitcast_f32(int32(round(x*A + B))), A=2^23/ln2, B=127*2^23-C
EXP_A = (1 << 23) / float(np.log(2.0))
EXP_C = 366500.0
EXP_B = float(127 * (1 << 23)) - EXP_C
# kpairs per unit offloaded off the ACT engine (engine alternates per unit)
OFF_KPAIRS = ()

WARMUP_MM = 12

bf16 = dt.bfloat16


def _bc(ap, idx, count):
    """Insert a broadcast (step 0) free dim at position idx of an AP."""
    a = list(ap.ap)
    a.insert(idx, [0, count])
    return bass.AP(tensor=ap.tensor, offset=ap.offset, ap=a)


def build_program():
    nc = bacc.Bacc(None, target_bir_lowering=False)

    xT = nc.dram_tensor("xT", [C, N], bf16, kind="ExternalInput")
    # host layout: [q0|k0 (256) | v (384) | q1|k1 | q2|k2]
    wqkvT = nc.dram_tensor("wqkvT", [C, 3 * HPC * D], bf16, kind="ExternalInput")
    projT = nc.dram_tensor("projT", [HPC * D, C], bf16, kind="ExternalInput")
    # tables pre-shuffled on host into the on-chip [partition, t*qk*d] layout
    # so the DMA is a contiguous full-rate transfer
    cqk = nc.dram_tensor("cqk", [128, NT * 2 * D], bf16, kind="ExternalInput")
    sqk = nc.dram_tensor("sqk", [128, NT * 2 * D], bf16, kind="ExternalInput")
    out = nc.dram_tensor("out", [N, C], dt.float32, kind="ExternalOutput")
    wsink_d = nc.dram_tensor("wsink", [128, 4], dt.float32, kind="ExternalOutput")

    with tile.TileContext(nc) as tc:
        with (
            tc.tile_pool(name="persist", bufs=1) as persist,
            tc.tile_pool(name="qkrot", bufs=2) as qkrot,     # qT/kT rotate across pairs
            tc.tile_pool(name="work", bufs=3) as work,
            tc.tile_pool(name="qkblk", bufs=4) as qkblk,
            tc.tile_pool(name="tiny", bufs=3) as tiny,
            tc.tile_pool(name="den", bufs=3) as den,
            tc.tile_pool(name="p2e", bufs=4) as p2e,
            tc.tile_pool(name="outp", bufs=4) as outp,
            tc.tile_pool(name="psS", bufs=2, space="PSUM") as psS,   # scores (2 banks ea)
            tc.tile_pool(name="psV", bufs=2, space="PSUM") as psV,   # AV accum
        ):
            psPrep_cm = tc.tile_pool(name="psPrep", bufs=2, space="PSUM")
            psPrep = psPrep_cm.__enter__()

            # ---------------- persistent tiles --------------------------------
            # PE p-state warmup first: junk matmuls on a zeroed tile keep the
            # PE continuously busy from t~0 so the first real matmul runs at
            # full clock (3us continuous-busy threshold in the ramp model).
            wscr = persist.tile([128, 256], bf16, tag="wscr")
            nc.vector.memset(wscr[:], 0.0)
            wpW = psPrep.tile([128, 384], dt.float32, tag="qkp")
            for _ in range(WARMUP_MM):
                nc.tensor.matmul(wpW[:, 0:256], wscr[:, 0:128], wscr[:], start=True, stop=True)
            # sink read so the BIR verifier sees a consumer of the warmup psum
            # (gpsimd cannot touch PSUM on hardware -- DVE does the read)
            wsink = persist.tile([128, 4], dt.float32, tag="wsink")
            nc.vector.tensor_copy(wsink[:], wpW[:, 0:4])
            nc.sync.dma_start(wsink_d[:, :], wsink[:])

            oT = [[persist.tile([128, G], bf16, name=f"oT{p}_{g}", tag=f"oT{p}_{g}")
                   for g in range(NG)] for p in range(3)]
            # v rows 0:64; cols 64:128 hold ones so the AV matmul emits the
            # softmax denominator already broadcast across 64 partitions
            # (matmul cost is free-size only -- the wider stationary is free)
            vA = [persist.tile([128, 4, HPC, 2 * D], dt.float32r, name=f"vA{kg}", tag=f"vA{kg}")
                  for kg in range(NG)]
            ones1 = persist.tile([128, 1], dt.float32, tag="ones1")
            nc.vector.memset(ones1[:], 1.0)
            for kg in range(NG):
                eng = nc.vector if kg % 2 == 0 else nc.gpsimd
                eng.tensor_copy(vA[kg][:, :, :, D : 2 * D],
                                _bc(_bc(_bc(ones1[:], 1, 4), 2, HPC), 3, D))
            ident = persist.tile([128, 128], bf16, tag="ident")
            make_identity(nc, ident[:])

            # weights / x^T / tables.  DMA order tuned so the first prep tiles
            # wait for the minimum byte set: pair-0 qk + v weights (wrA), then
            # x token-group 0, tables, the rest of x, pair-1/2 weights, proj.
            xw_cm = tc.tile_pool(name="xw", bufs=1)
            xw = xw_cm.__enter__()
            # one merged DMA per logical input block: the HWDGE dispatcher is
            # a serialized 625ns/DMA resource while transfers run at full
            # aggregate bandwidth, so fewer+bigger DMAs reach the PE sooner.
            xTr = xT.rearrange("(j p) n -> p j n", p=128)
            wqr = wqkvT.rearrange("(j p) c -> p j c", p=128)
            # v-half of the qkv weights first so the lead V matmuls (tile 0)
            # start as early as possible; x group-0 in two halves so chunk
            # j=0..2 land before the full transfer completes
            wrA_t = xw.tile([128, NCHUNK, 640], bf16, name="wrA", tag="wrA")
            nc.sync.dma_start(wrA_t[:, :, 256:640], wqr[:, :, 256:640])
            xr_t = [xw.tile([128, NCHUNK, G], bf16, name=f"xr{tg}", tag=f"xr{tg}")
                    for tg in range(NG)]
            nc.sync.dma_start(xr_t[0][:, 0:3, :], xTr[:, 0:3, 0:G])
            nc.sync.dma_start(xr_t[0][:, 3:6, :], xTr[:, 3:6, 0:G])
            nc.sync.dma_start(wrA_t[:, :, 0:256], wqr[:, :, 0:256])
            wrA = [wrA_t[:, j, :] for j in range(NCHUNK)]
            xr = [[xr_t[tg][:, j, :] for tg in range(NG)] for j in range(NCHUNK)]
            tabs = {}
            for name, dram in (("cqk", cqk), ("sqk", sqk)):
                t = persist.tile([128, NT, 2, D], bf16, name=name, tag=name)
                nc.sync.dma_start(t[:].rearrange("p t qk d -> p (t qk d)"), dram[:, :])
                tabs[name] = t
            for tg in range(1, NG):
                nc.sync.dma_start(xr_t[tg][:], xTr[:, :, tg * G : (tg + 1) * G])
            wrB_t = xw.tile([128, NCHUNK, 512], bf16, name="wrB", tag="wrB")
            nc.sync.dma_start(wrB_t[:], wqr[:, :, 640:1152])
            wrB = [wrB_t[:, j, :] for j in range(NCHUNK)]
            prW_t = persist.tile([128, 3, C], bf16, name="prW", tag="prW")
            nc.sync.dma_start(prW_t[:], projT.rearrange("(p q) c -> q p c", q=128))
            prW = [prW_t[:, p, :] for p in range(3)]

            # ---------------- prep: qkv -> rmsnorm -> rope -> qkT -------------
            def new_pair_state(p):
                # qkT[g]: [128 feat, 2, G]: [:,0,:] = q^T group g, [:,1,:] = k^T
                # (feat rows: head hh at 64*hh..64*hh+64)
                return {
                    "p": p,
                    "qkT": [qkrot.tile([128, 2, G], bf16, name=f"qkT{p}_{g}", tag=f"qkT{g}")
                            for g in range(NG)],
                }

            def emit_prep_chain(st, i, qkp, vp=None, p0=False):
                # in the pair-0 head the ACT engine is mostly idle: use it for
                # the psum->sbuf copies so Pool/DVE keep pace with the PE
                qk_sb = qkblk.tile([128, 256], bf16, tag="qk_sb")
                if p0:
                    nc.scalar.copy(qk_sb[:], qkp[:, 0:256])
                else:
                    nc.vector.tensor_copy(qk_sb[:], qkp[:, 0:256])
                # DVE: sum of squares per (q0,q1,k0,k1) signal
                sq = work.tile([128, 256], bf16, tag="sq")
                nc.vector.tensor_tensor(sq[:], qk_sb[:], qk_sb[:], op=ALU.mult)
                ss = tiny.tile([128, 4], dt.float32, tag="ss16")
                nc.vector.tensor_reduce(ss[:], sq[:].rearrange("p (h d) -> p h d", h=4),
                                        axis=AX.X, op=ALU.add)
                # rsqrt on DVE (bit-trick + 2 Newton): nf = 1/sqrt(ss+D*EPS)
                ssh = tiny.tile([128, 4], dt.float32, tag="ssh")
                nc.vector.tensor_scalar(ssh[:], ss[:], 0.5, 0.5 * D * EPS,
                                        op0=ALU.mult, op1=ALU.add)
                y0i = tiny.tile([128, 4], dt.int32, tag="y0i")
                nc.vector.tensor_scalar(y0i[:], ss[:].bitcast(dt.int32), 1, 0,
                                        op0=ALU.logical_shift_right, op1=ALU.bitwise_or)
                nc.vector.tensor_scalar(y0i[:], y0i[:], -1, 0x5F3759DF,
                                        op0=ALU.mult, op1=ALU.add)
                nf16 = tiny.tile([128, 4], dt.float32, tag="nf16")
                y1 = tiny.tile([128, 4], dt.float32, tag="y1")
                yw = tiny.tile([128, 4], dt.float32, tag="yw")
                y = y0i[:].bitcast(dt.float32)
                for dst_ in (y1, nf16):
                    nc.vector.tensor_tensor(yw[:], y, y, op=ALU.mult)
                    nc.vector.tensor_tensor(yw[:], yw[:], ssh[:], op=ALU.mult)
                    nc.vector.tensor_scalar(yw[:], yw[:], -1.0, 1.5,
                                            op0=ALU.mult, op1=ALU.add)
                    nc.vector.tensor_tensor(dst_[:], y, yw[:], op=ALU.mult)
                    y = dst_[:]
                # rope (tables fold x8 and the norm weights); norm scale last
                t4 = qk_sb[:].rearrange("p (qk h d) -> p qk h d", qk=2, h=2)
                cwb = _bc(tabs["cqk"][:, i, :, :], 2, 2)
                swb = _bc(tabs["sqk"][:, i, :, :], 2, 2)
                m1 = work.tile([128, 2, 2, D], bf16, tag="m1")
                nc.gpsimd.tensor_tensor(m1[:], t4, cwb, op=ALU.mult)
                m2 = work.tile([128, 2, 2, D], bf16, tag="m2")
                h_ = D // 2
                nc.vector.tensor_tensor(m2[:, :, :, 0:h_], t4[:, :, :, h_:D],
                                        swb[:, :, :, 0:h_], op=ALU.mult)
                nc.vector.tensor_tensor(m2[:, :, :, h_:D], t4[:, :, :, 0:h_],
                                        swb[:, :, :, h_:D], op=ALU.mult)
                qn = work.tile([128, 2, 2, D], bf16, tag="qn", bufs=4)
                nc.vector.tensor_tensor(qn[:], m1[:], m2[:], op=ALU.add)
                for qk in range(2):
                    for h2 in range(2):
                        s_ = 2 * qk + h2
                        nc.vector.tensor_scalar(qn[:, qk, h2, :], qn[:, qk, h2, :],
                                                nf16[:, s_ : s_ + 1], None, op0=ALU.mult)
                # Feature-major transposes into qkT.  Pair-0 (the head) uses
                # low-latency PE transposes (bf16 identity, psum staging in
                # the qkp tile tail); later pairs have a whole pair of slack,
                # so the XBAR DMA engines do the transpose off the PE and
                # write qkT directly (no psum, no copy).
                qn2 = qn[:].rearrange("p a h d -> p (a h d)")
                dst = st["qkT"][i // NG]
                if not p0:
                    nc.sync.dma_start_transpose(
                        dst[:, 0, (i % NG) * 128 : (i % NG + 1) * 128], qn2[:, 0:128])
                    nc.sync.dma_start_transpose(
                        dst[:, 1, (i % NG) * 128 : (i % NG + 1) * 128], qn2[:, 128:256])
                    return lambda: None
                tp = qkp[:, 256:384].bitcast(bf16)

                def emit_transposes():
                    nc.tensor.transpose(tp[:, 0:128], qn2[:, 0:128], ident[:])
                    nc.tensor.transpose(tp[:, 128:256], qn2[:, 128:256], ident[:])
                    dslice = dst[:, :, (i % NG) * 128 : (i % NG + 1) * 128]
                    tsl = tp.rearrange("p (qk c) -> p qk c", qk=2)
                    nc.vector.tensor_copy(dslice, tsl)
                return emit_transposes

            def p0_v_tile(i):
                # V psum borrows a scores-ring slot (full-size alloc, sliced).
                # V matmuls lead the qk stream by 2 tiles so their psum slots
                # allocate before the gated attention's exp pressure builds.
                vp_t = psS.tile([128, 1024], dt.float32, tag="sp")
                vp = vp_t[:, 0 : HPC * D]
                for j in range(NCHUNK):
                    nc.tensor.matmul(vp, xr[j][i // NG][:, (i % NG) * 128 : (i % NG + 1) * 128],
                                     wrA[j][:, 256:640],
                                     start=(j == 0), stop=(j == NCHUNK - 1))
                nc.scalar.copy(vA[i // NG][:, i % NG, :, 0:D],
                               vp.rearrange("p (h d) -> p h d", h=HPC))

            def p0_qk_tile(st, i, pend):
                if len(pend) >= 2:
                    pend.pop(0)()
                qkp = psPrep.tile([128, 384], dt.float32, tag="qkp")
                for j in range(NCHUNK):
                    nc.tensor.matmul(qkp[:, 0:256],
                                     xr[j][i // NG][:, (i % NG) * 128 : (i % NG + 1) * 128],
                                     wrA[j][:, 0:256],
                                     start=(j == 0), stop=(j == NCHUNK - 1))
                pend.append(emit_prep_chain(st, i, qkp, None, p0=True))

            def pair_prep_quanta(st):
                """Pair p>0 prep as PE quanta: per tile 3x (2 matmuls) then
                the vector chain; each tile's transposes are deferred into
                the middle of the NEXT tile so the PE never waits on the
                DVE chain."""
                p = st["p"]
                pend_tr = None
                for i in range(NT):
                    qkp = psPrep.tile([128, 384], dt.float32, tag="qkp")
                    for j3 in range(3):
                        for j in (2 * j3, 2 * j3 + 1):
                            nc.tensor.matmul(qkp[:, 0:256],
                                             xr[j][i // NG][:, (i % NG) * 128 : (i % NG + 1) * 128],
                                             wrB[j][:, (p - 1) * 256 : p * 256],
                                             start=(j == 0), stop=(j == NCHUNK - 1))
                        yield
                    pend_tr_new = emit_prep_chain(st, i, qkp)
                    if pend_tr is not None:
                        pend_tr()
                    pend_tr = pend_tr_new
                    yield
                if pend_tr is not None:
                    pend_tr()

            # ---------------- attention ---------------------------------------
            def att_unit_gen(st, g, hh, off_engine):
                p = st["p"]
                h = 2 * p + hh
                off = 64 * hh
                av = psV.tile([128, G], dt.float32, tag="av")

                def emit_av(kpair, es):
                    for half in range(2):
                        ki = kpair * 2 + half
                        nc.tensor.matmul(
                            av[:],
                            vA[ki // NG][:, ki % NG, h, :],
                            es[:, half * 512 : (half + 1) * 512],
                            start=(ki == 0), stop=(ki == NT - 1),
                        )

                pend = []
                for kpair in range(8):
                    # filler slot FIRST so interleaved prep/proj matmuls run
                    # while the sp-slot / es semaphores settle.
                    yield
                    sp = psS.tile([128, 1024], dt.float32, tag="sp")
                    for half in range(2):
                        ki = kpair * 2 + half
                        nc.tensor.matmul(
                            sp[:, half * 512 : (half + 1) * 512],
                            st["qkT"][ki // NG][off : off + 64, 1,
                                                (ki % NG) * 128 : (ki % NG + 1) * 128],
                            st["qkT"][g][off : off + 64, 0, :],
                            start=True, stop=True,
                        )
                    es = p2e.tile([128, 1024], dt.float32r, name="est", tag="es")
                    nc.scalar.activation(es[:], sp[:], AF.Exp, scale=SCALE)
                    pend.append((kpair, es))
                    if len(pend) > 2:
                        emit_av(*pend.pop(0))
                while pend:
                    emit_av(*pend.pop(0))
                rd = den.tile([64, G], dt.float32, tag="rd")
                nc.vector.reciprocal(rd[:], av[64:128, :])
                nc.vector.tensor_tensor(
                    oT[p][g][off : off + 64, :],
                    av[0:64, :], rd[:], op=ALU.mult,
                )

            # ---------------- projection --------------------------------------
            psProj = None

            def proj_tile_quanta(i):
                if i >= 12 and i % 2 == 0:
                    # tail tiles: the attention score ring is idle by now --
                    # borrow its slots so the last tiles don't serialize on
                    # the single psProj buffer
                    pj_t = psS.tile([128, 1024], dt.float32, name="pjsp", tag="sp")
                    pj = pj_t[:, 0:C]
                else:
                    pj_t = psProj.tile([128, C], dt.float32, name="pj", tag="pj", bufs=1)
                    pj = pj_t[:]
                for pp_ in range(3):
                    st_, spp = (pp_ == 0), (pp_ == 2)
                    sl = oT[pp_][i // NG][:, (i % NG) * 128 : (i % NG + 1) * 128]
                    nc.tensor.matmul(pj[:, 0:512], sl, prW[pp_][:, 0:512], start=st_, stop=spp)
                    nc.tensor.matmul(pj[:, 512:768], sl, prW[pp_][:, 512:768], start=st_, stop=spp)
                    yield
                os_ = outp.tile([128, C], dt.float32, tag="os")
                # split the copy across engines: shortens the pj-ring latency;
                # each half's writeback launches as soon as its copy lands
                if i >= 12:
                    nc.vector.tensor_copy(os_[:, 0:384], pj[:, 0:384])
                    nc.sync.dma_start(out[i * 128 : (i + 1) * 128, 0:384], os_[:, 0:384])
                    nc.scalar.copy(os_[:, 384:768], pj[:, 384:768])
                    nc.sync.dma_start(out[i * 128 : (i + 1) * 128, 384:768], os_[:, 384:768])
                else:
                    nc.vector.tensor_copy(os_[:], pj[:])
                    nc.sync.dma_start(out[i * 128 : (i + 1) * 128, :], os_[:])
                yield

            # ---------------- schedule ----------------------------------------
            def off_eng(unit_idx):
                return "dve" if unit_idx % 2 == 0 else "pool"

            # pair-0 prep up front; the first two attention units' (g=0,
            # hh=0/1) kpairs are gated in as their kT tiles are transposed,
            # interleaved so no single stalled kpair blocks the PE stream.
            cur = new_pair_state(0)
            hgen = [att_unit_gen(cur, 0, 0, off_eng(0)),
                    att_unit_gen(cur, 0, 1, off_eng(1))]
            for hg in hgen:
                next(hg, None)  # advance to the first slot boundary
            # gate kpair j only after the transposes of its kT tiles (2j+1,
            # deferred 2 tiles -> emitted at tile 2j+3) are in the stream
            gates = [(5, 0), (5, 0), (6, 1), (7, 0), (7, 1), (9, 0), (9, 1),
                     (11, 0), (11, 1), (13, 0), (13, 1), (15, 0)]
            pend = []
            p0_v_tile(0)
            p0_v_tile(1)
            for i in range(NT):
                if i + 2 < NT:
                    p0_v_tile(i + 2)
                p0_qk_tile(cur, i, pend)
                while gates and i >= gates[0][0]:
                    next(hgen[gates.pop(0)[1]], None)
            pend.pop(0)()          # tile 14 transposes
            next(hgen[1], None)    # u01 kp5
            pend.pop(0)()          # tile 15 transposes
            next(hgen[1], None)    # u01 kp6
            for hg in hgen:
                for _ in hg:
                    pass

            unit_idx = 2
            proj_state = {"g": 0, "queue": list(range(NT)), "gen": None}
            _SENT = object()

            def pump_proj(n):
                for _ in range(n):
                    if proj_state["gen"] is not None:
                        if next(proj_state["gen"], _SENT) is not _SENT:
                            continue
                        proj_state["gen"] = None
                    if proj_state["queue"] and proj_state["queue"][0] // NG < proj_state["g"]:
                        proj_state["gen"] = proj_tile_quanta(proj_state["queue"].pop(0))
                        next(proj_state["gen"], _SENT)
                    else:
                        return


            filler = None
            for p in range(3):
                if p < 2:
                    nxt = new_pair_state(p + 1)
                    filler = pair_prep_quanta(nxt)
                    n_units = 6 if p == 0 else 8
                    fill_state = {"rate": 4 * NT / (8 * n_units), "acc": 0.0}
                else:
                    nxt = None
                    psPrep_cm.__exit__(None, None, None)
                    psProj_cm = tc.tile_pool(name="psProj", bufs=1, space="PSUM")
                    psProj = psProj_cm.__enter__()
                for g in range(NG):
                    if p == 2:
                        proj_state["g"] = g
                    for hh in range(2):
                        if p == 0 and g == 0:
                            continue
                        gen = att_unit_gen(cur, g, hh, off_eng(unit_idx))
                        unit_idx += 1
                        for _ in gen:
                            if p < 2:
                                # fractional pacing: emit quanta at a steady
                                # rate so prep neither lags nor bursts ahead
                                fill_state["acc"] += fill_state["rate"]
                                while fill_state["acc"] >= 1.0:
                                    fill_state["acc"] -= 1.0
                                    next(filler, _SENT)
                            else:
                                pump_proj(1)
                                proj_state["acc"] = proj_state.get("acc", 0.0) + 0.25
                                if proj_state["acc"] >= 1.0:
                                    proj_state["acc"] -= 1.0
                                    pump_proj(1)
                if p < 2:
                    for _ in filler:
                        pass
                    cur = nxt
            proj_state["g"] = NG
            # drain order: pj-ring tile first (no sp-ring wait), then the
            # sp-borrowing tiles while the last exps retire, pj tile last
            if proj_state["queue"] == [12, 13, 14, 15]:
                proj_state["queue"] = [13, 12, 14, 15]
            while proj_state["queue"] or proj_state["gen"] is not None:
                pump_proj(1)
            psProj_cm.__exit__(None, None, None)
            xw_cm.__exit__(None, None, None)

    nc.compile()
    return nc


_NC = None


def _get_nc():
    global _NC
    if _NC is None:
        _NC = build_program()
    return _NC


def _prep_inputs(x, cos, sin, qkv_w, q_norm_w, k_norm_w, proj_w):
    cos2 = np.asarray(cos, np.float32).reshape(N, D // 2)
    sin2 = np.asarray(sin, np.float32).reshape(N, D // 2)
    cos_full = np.concatenate([cos2, cos2], axis=1)          # [N, 64]
    sin_signed = np.concatenate([-sin2, sin2], axis=1)       # [N, 64]

    def tables(w):
        w = np.asarray(w, np.float32)
        wswap = np.concatenate([w[D // 2 :], w[: D // 2]])
        cw = (8.0 * cos_full * w[None, :]).astype(np.float32)
        sw = (8.0 * sin_signed * wswap[None, :]).astype(np.float32)
        return cw, sw

    cwq_, swq_ = tables(q_norm_w)
    cwk_, swk_ = tables(k_norm_w)

    def shuffle_tab(a):
        # [N, 2D] -> on-chip [partition 128, (t, qk, d)] layout, contiguous
        a = a.reshape(N // 128, 128, 2 * D).transpose(1, 0, 2).reshape(128, -1)
        return np.ascontiguousarray(a.astype(ml_dtypes.bfloat16))

    cqk_ = shuffle_tab(np.stack([cwq_, cwk_], axis=1).reshape(N, 2 * D))
    sqk_ = shuffle_tab(np.stack([swq_, swk_], axis=1).reshape(N, 2 * D))

    in_maps = []
    for c in range(8):
        b, hg = c // 2, c % 2
        h0 = HPC * hg
        rows = np.r_[h0 * D : (h0 + HPC) * D]
        wq = qkv_w[rows]          # [384, C]
        wk = qkv_w[C + rows]
        wv = qkv_w[2 * C + rows]
        # pack as [q0|k0 (256), v (384), q1|k1, q2|k2]
        parts = [wq[0:128], wk[0:128], wv]
        for p in range(1, 3):
            parts.append(wq[p * 128 : (p + 1) * 128])
            parts.append(wk[p * 128 : (p + 1) * 128])
        wqkvT_ = np.ascontiguousarray(
            np.concatenate(parts, 0).T.astype(ml_dtypes.bfloat16))
        projT_ = np.ascontiguousarray(
            proj_w[:, rows].T.astype(ml_dtypes.bfloat16))
        xT_ = np.ascontiguousarray(x[b].T.astype(ml_dtypes.bfloat16))
        in_maps.append({
            "xT": xT_, "wqkvT": wqkvT_, "projT": projT_,
            "cqk": cqk_, "sqk": sqk_,
        })
    return in_maps


def kernel(x, cos, sin, qkv_w, q_norm_w, k_norm_w, proj_w, proj_b, _want_trace=False):
    x = np.asarray(x, np.float32)
    qkv_w = np.asarray(qkv_w, np.float32)
    proj_w = np.asarray(proj_w, np.float32)
    proj_b = np.asarray(proj_b, np.float32)
    in_maps = _prep_inputs(x, cos, sin, qkv_w, q_norm_w, k_norm_w, proj_w)
    nc = _get_nc()
    res = run_bass_kernel_spmd(nc, in_maps, core_ids=list(range(8)), trace=_want_trace)
    out = np.empty((B, N, C), np.float32)
    for b in range(B):
        out[b] = res.results[2 * b]["out"] + res.results[2 * b + 1]["out"] + proj_b[None, :]
    if _want_trace:
        return out, res
    return out


# revision 104
# speedup vs baseline: 1.0004x; 1.0004x over previous
"""Trainium2 Bass kernel for nn_Attention (B=4, N=2048, C=768, H=12, D=64).

Sharding: core c -> batch b=c//2, head-group hg=c%2 (6 heads each).
qkv_w column-parallel, proj_w row-parallel (host sums the 2 partials per b).

v2 design notes (vs the fp32r baseline):
  - All weight/activation matmul operands are bf16 (qkv, scores, proj);
    exp outputs (es) and V (vA) stay fp32r so the AV matmuls are pure fp32r.
    Matmul cost is 1.0 cycles/row either way; bf16 buys SBUF room, DVE
    2x/4x modes, and XBAR DMA transposes (which need a 2-byte dtype).
  - q/k feature-major transposes are hybrid: the latency-critical pair-0
    head uses PE transposes (bf16 identity, staged in the qkp psum tail);
    pairs 1-2 prep has a whole pair of slack, so the XBAR DMA engines
    transpose straight into qkT, off the PE and with no psum or copy.
  - The Activation engine runs only the softmax exp plus a few head-time
    copies; all other psum->sbuf traffic is on DVE (gpsimd cannot access
    PSUM on real hardware, although the cost model allows it).
  - vA carries a 64-wide ones block so each AV matmul emits the softmax
    denominator already broadcast across partitions (matmul cost depends
    only on the moving free size, so the wider stationary is free); the
    normalize is then just reciprocal + multiply on DVE.
  - Prep (for the next head-pair) and projection are emitted in small PE
    quanta between attention kpairs, rate-paced so the PE stream has no
    long stalls; transposes are deferred ~1-2 tiles behind their chain so
    the PE never waits on the DVE rope/norm chain.
  - Inputs arrive as few merged DMAs (the HWDGE dispatcher serializes at
    625ns/DMA while transfers run at full aggregate bandwidth), tables are
    host-pre-shuffled into the on-chip layout for a full-rate contiguous
    transfer, and junk matmuls at t~0 keep the PE p-state ramp warm.
"""
import sys

sys.path.insert(0, "/opt/trn_rl_repo")

import ml_dtypes
import numpy as np
import concourse.bass as bass
import concourse.mybir as mybir
import concourse.tile as tile
from concourse import bacc
from concourse.bass_utils import run_bass_kernel_spmd
from concourse.masks import make_identity

dt = mybir.dt
AF = mybir.ActivationFunctionType
ALU = mybir.AluOpType
AX = mybir.AxisListType

B, N, C = 4, 2048, 768
H, D = 12, 64
HPC = 6            # heads per core
EPS = 1e-6
NT = N // 128      # 16 token tiles
NCHUNK = C // 128  # 6 contraction chunks
SCALE = D ** -0.5  # 0.125
NG = 4             # qi groups
G = N // NG        # 512 per group

# Schraudolph exp constants (round-to-nearest fp32->int32 convert):
# exp(x) ~= bitcast_f32(int32(round(x*A + B))), A=2^23/ln2, B=127*2^23-C
EXP_A = (1 << 23) / float(np.log(2.0))
EXP_C = 366500.0
EXP_B = float(127 * (1 << 23)) - EXP_C
# kpairs per unit offloaded off the ACT engine (engine alternates per unit)
OFF_KPAIRS = ()

WARMUP_MM = 11

bf16 = dt.bfloat16


def _bc(ap, idx, count):
    """Insert a broadcast (step 0) free dim at position idx of an AP."""
    a = list(ap.ap)
    a.insert(idx, [0, count])
    return bass.AP(tensor=ap.tensor, offset=ap.offset, ap=a)


def build_program():
    nc = bacc.Bacc(None, target_bir_lowering=False)

    xT = nc.dram_tensor("xT", [C, N], bf16, kind="ExternalInput")
    # host layout: [q0|k0 (256) | v (384) | q1|k1 | q2|k2]
    wqkvT = nc.dram_tensor("wqkvT", [C, 3 * HPC * D], bf16, kind="ExternalInput")
    projT = nc.dram_tensor("projT", [HPC * D, C], bf16, kind="ExternalInput")
    # tables pre-shuffled on host into the on-chip [partition, t*qk*d] layout
    # so the DMA is a contiguous full-rate transfer
    cqk = nc.dram_tensor("cqk", [128, NT * 2 * D], bf16, kind="ExternalInput")
    sqk = nc.dram_tensor("sqk", [128, NT * 2 * D], bf16, kind="ExternalInput")
    out = nc.dram_tensor("out", [N, C], dt.float32, kind="ExternalOutput")
    wsink_d = nc.dram_tensor("wsink", [128, 4], dt.float32, kind="ExternalOutput")

    with tile.TileContext(nc) as tc:
        with (
            tc.tile_pool(name="persist", bufs=1) as persist,
            tc.tile_pool(name="qkrot", bufs=2) as qkrot,     # qT/kT rotate across pairs
            tc.tile_pool(name="work", bufs=3) as work,
            tc.tile_pool(name="qkblk", bufs=4) as qkblk,
            tc.tile_pool(name="tiny", bufs=3) as tiny,
            tc.tile_pool(name="den", bufs=3) as den,
            tc.tile_pool(name="p2e", bufs=4) as p2e,
            tc.tile_pool(name="outp", bufs=4) as outp,
            tc.tile_pool(name="psS", bufs=2, space="PSUM") as psS,   # scores (2 banks ea)
            tc.tile_pool(name="psV", bufs=2, space="PSUM") as psV,   # AV accum
        ):
            psPrep_cm = tc.tile_pool(name="psPrep", bufs=2, space="PSUM")
            psPrep = psPrep_cm.__enter__()

            # ---------------- persistent tiles --------------------------------
            # PE p-state warmup first: junk matmuls on a zeroed tile keep the
            # PE continuously busy from t~0 so the first real matmul runs at
            # full clock (3us continuous-busy threshold in the ramp model).
            wscr = persist.tile([128, 256], bf16, tag="wscr")
            nc.vector.memset(wscr[:], 0.0)
            wpW = psPrep.tile([128, 384], dt.float32, tag="qkp")
            for _ in range(WARMUP_MM):
                nc.tensor.matmul(wpW[:, 0:256], wscr[:, 0:128], wscr[:], start=True, stop=True)
            # sink read so the BIR verifier sees a consumer of the warmup psum
            # (gpsimd cannot touch PSUM on hardware -- DVE does the read)
            wsink = persist.tile([128, 4], dt.float32, tag="wsink")
            nc.vector.tensor_copy(wsink[:], wpW[:, 0:4])
            nc.sync.dma_start(wsink_d[:, :], wsink[:])

            oT = [[persist.tile([128, G], bf16, name=f"oT{p}_{g}", tag=f"oT{p}_{g}")
                   for g in range(NG)] for p in range(3)]
            # v rows 0:64; cols 64:128 hold ones so the AV matmul emits the
            # softmax denominator already broadcast across 64 partitions
            # (matmul cost is free-size only -- the wider stationary is free)
            vA = [persist.tile([128, 4, HPC, 2 * D], dt.float32r, name=f"vA{kg}", tag=f"vA{kg}")
                  for kg in range(NG)]
            ones1 = persist.tile([128, 1], dt.float32, tag="ones1")
            nc.vector.memset(ones1[:], 1.0)
            for kg in range(NG):
                eng = nc.vector if kg % 2 == 0 else nc.gpsimd
                eng.tensor_copy(vA[kg][:, :, :, D : 2 * D],
                                _bc(_bc(_bc(ones1[:], 1, 4), 2, HPC), 3, D))
            ident = persist.tile([128, 128], bf16, tag="ident")
            make_identity(nc, ident[:])

            # weights / x^T / tables.  DMA order tuned so the first prep tiles
            # wait for the minimum byte set: pair-0 qk + v weights (wrA), then
            # x token-group 0, tables, the rest of x, pair-1/2 weights, proj.
            xw_cm = tc.tile_pool(name="xw", bufs=1)
            xw = xw_cm.__enter__()
            # one merged DMA per logical input block: the HWDGE dispatcher is
            # a serialized 625ns/DMA resource while transfers run at full
            # aggregate bandwidth, so fewer+bigger DMAs reach the PE sooner.
            xTr = xT.rearrange("(j p) n -> p j n", p=128)
            wqr = wqkvT.rearrange("(j p) c -> p j c", p=128)
            # v-half of the qkv weights first so the lead V matmuls (tile 0)
            # start as early as possible; x group-0 in two halves so chunk
            # j=0..2 land before the full transfer completes
            wrA_t = xw.tile([128, NCHUNK, 640], bf16, name="wrA", tag="wrA")
            nc.sync.dma_start(wrA_t[:, :, 256:640], wqr[:, :, 256:640])
            xr_t = [xw.tile([128, NCHUNK, G], bf16, name=f"xr{tg}", tag=f"xr{tg}")
                    for tg in range(NG)]
            nc.sync.dma_start(xr_t[0][:, 0:3, :], xTr[:, 0:3, 0:G])
            nc.sync.dma_start(xr_t[0][:, 3:6, :], xTr[:, 3:6, 0:G])
            nc.sync.dma_start(wrA_t[:, :, 0:256], wqr[:, :, 0:256])
            wrA = [wrA_t[:, j, :] for j in range(NCHUNK)]
            xr = [[xr_t[tg][:, j, :] for tg in range(NG)] for j in range(NCHUNK)]
            tabs = {}
            for name, dram in (("cqk", cqk), ("sqk", sqk)):
                t = persist.tile([128, NT, 2, D], bf16, name=name, tag=name)
                nc.sync.dma_start(t[:].rearrange("p t qk d -> p (t qk d)"), dram[:, :])
                tabs[name] = t
            for tg in range(1, NG):
                nc.sync.dma_start(xr_t[tg][:], xTr[:, :, tg * G : (tg + 1) * G])
            wrB_t = xw.tile([128, NCHUNK, 512], bf16, name="wrB", tag="wrB")
            nc.sync.dma_start(wrB_t[:], wqr[:, :, 640:1152])
            wrB = [wrB_t[:, j, :] for j in range(NCHUNK)]
            prW_t = persist.tile([128, 3, C], bf16, name="prW", tag="prW")
            nc.sync.dma_start(prW_t[:], projT.rearrange("(p q) c -> q p c", q=128))
            prW = [prW_t[:, p, :] for p in range(3)]

            # ---------------- prep: qkv -> rmsnorm -> rope -> qkT -------------
            def new_pair_state(p):
                # qkT[g]: [128 feat, 2, G]: [:,0,:] = q^T group g, [:,1,:] = k^T
                # (feat rows: head hh at 64*hh..64*hh+64)
                return {
                    "p": p,
                    "qkT": [qkrot.tile([128, 2, G], bf16, name=f"qkT{p}_{g}", tag=f"qkT{g}")
                            for g in range(NG)],
                }

            def emit_prep_chain(st, i, qkp, vp=None, p0=False):
                # in the pair-0 head the ACT engine is mostly idle: use it for
                # the psum->sbuf copies so Pool/DVE keep pace with the PE
                qk_sb = qkblk.tile([128, 256], bf16, tag="qk_sb")
                if p0:
                    nc.scalar.copy(qk_sb[:], qkp[:, 0:256])
                else:
                    nc.vector.tensor_copy(qk_sb[:], qkp[:, 0:256])
                # DVE: sum of squares per (q0,q1,k0,k1) signal
                sq = work.tile([128, 256], bf16, tag="sq")
                nc.vector.tensor_tensor(sq[:], qk_sb[:], qk_sb[:], op=ALU.mult)
                ss = tiny.tile([128, 4], dt.float32, tag="ss16")
                nc.vector.tensor_reduce(ss[:], sq[:].rearrange("p (h d) -> p h d", h=4),
                                        axis=AX.X, op=ALU.add)
                # rsqrt on DVE (bit-trick + 2 Newton): nf = 1/sqrt(ss+D*EPS)
                ssh = tiny.tile([128, 4], dt.float32, tag="ssh")
                nc.vector.tensor_scalar(ssh[:], ss[:], 0.5, 0.5 * D * EPS,
                                        op0=ALU.mult, op1=ALU.add)
                y0i = tiny.tile([128, 4], dt.int32, tag="y0i")
                nc.vector.tensor_scalar(y0i[:], ss[:].bitcast(dt.int32), 1, 0,
                                        op0=ALU.logical_shift_right, op1=ALU.bitwise_or)
                nc.vector.tensor_scalar(y0i[:], y0i[:], -1, 0x5F3759DF,
                                        op0=ALU.mult, op1=ALU.add)
                nf16 = tiny.tile([128, 4], dt.float32, tag="nf16")
                y1 = tiny.tile([128, 4], dt.float32, tag="y1")
                yw = tiny.tile([128, 4], dt.float32, tag="yw")
                y = y0i[:].bitcast(dt.float32)
                for dst_ in (y1, nf16):
                    nc.vector.tensor_tensor(yw[:], y, y, op=ALU.mult)
                    nc.vector.tensor_tensor(yw[:], yw[:], ssh[:], op=ALU.mult)
                    nc.vector.tensor_scalar(yw[:], yw[:], -1.0, 1.5,
                                            op0=ALU.mult, op1=ALU.add)
                    nc.vector.tensor_tensor(dst_[:], y, yw[:], op=ALU.mult)
                    y = dst_[:]
                # rope (tables fold x8 and the norm weights); norm scale last
                t4 = qk_sb[:].rearrange("p (qk h d) -> p qk h d", qk=2, h=2)
                cwb = _bc(tabs["cqk"][:, i, :, :], 2, 2)
                swb = _bc(tabs["sqk"][:, i, :, :], 2, 2)
                m1 = work.tile([128, 2, 2, D], bf16, tag="m1")
                nc.gpsimd.tensor_tensor(m1[:], t4, cwb, op=ALU.mult)
                m2 = work.tile([128, 2, 2, D], bf16, tag="m2")
                h_ = D // 2
                nc.vector.tensor_tensor(m2[:, :, :, 0:h_], t4[:, :, :, h_:D],
                                        swb[:, :, :, 0:h_], op=ALU.mult)
                nc.vector.tensor_tensor(m2[:, :, :, h_:D], t4[:, :, :, 0:h_],
                                        swb[:, :, :, h_:D], op=ALU.mult)
                qn = work.tile([128, 2, 2, D], bf16, tag="qn", bufs=4)
                nc.vector.tensor_tensor(qn[:], m1[:], m2[:], op=ALU.add)
                for qk in range(2):
                    for h2 in range(2):
                        s_ = 2 * qk + h2
                        nc.vector.tensor_scalar(qn[:, qk, h2, :], qn[:, qk, h2, :],
                                                nf16[:, s_ : s_ + 1], None, op0=ALU.mult)
                # Feature-major transposes into qkT.  Pair-0 (the head) uses
                # low-latency PE transposes (bf16 identity, psum staging in
                # the qkp tile tail); later pairs have a whole pair of slack,
                # so the XBAR DMA engines do the transpose off the PE and
                # write qkT directly (no psum, no copy).
                qn2 = qn[:].rearrange("p a h d -> p (a h d)")
                dst = st["qkT"][i // NG]
                if not p0:
                    nc.sync.dma_start_transpose(
                        dst[:, 0, (i % NG) * 128 : (i % NG + 1) * 128], qn2[:, 0:128])
                    nc.sync.dma_start_transpose(
                        dst[:, 1, (i % NG) * 128 : (i % NG + 1) * 128], qn2[:, 128:256])
                    return lambda: None
                tp = qkp[:, 256:384].bitcast(bf16)

                def emit_transposes():
                    nc.tensor.transpose(tp[:, 0:128], qn2[:, 0:128], ident[:])
                    nc.tensor.transpose(tp[:, 128:256], qn2[:, 128:256], ident[:])
                    dslice = dst[:, :, (i % NG) * 128 : (i % NG + 1) * 128]
                    tsl = tp.rearrange("p (qk c) -> p qk c", qk=2)
                    nc.vector.tensor_copy(dslice, tsl)
                return emit_transposes

            def p0_v_tile(i):
                # V psum borrows a scores-ring slot (full-size alloc, sliced).
                # V matmuls lead the qk stream by 2 tiles so their psum slots
                # allocate before the gated attention's exp pressure builds.
                vp_t = psS.tile([128, 1024], dt.float32, tag="sp")
                vp = vp_t[:, 0 : HPC * D]
                for j in range(NCHUNK):
                    nc.tensor.matmul(vp, xr[j][i // NG][:, (i % NG) * 128 : (i % NG + 1) * 128],
                                     wrA[j][:, 256:640],
                                     start=(j == 0), stop=(j == NCHUNK - 1))
                nc.scalar.copy(vA[i // NG][:, i % NG, :, 0:D],
                               vp.rearrange("p (h d) -> p h d", h=HPC))

            def p0_qk_tile(st, i, pend):
                if len(pend) >= 2:
                    pend.pop(0)()
                qkp = psPrep.tile([128, 384], dt.float32, tag="qkp")
                for j in range(NCHUNK):
                    nc.tensor.matmul(qkp[:, 0:256],
                                     xr[j][i // NG][:, (i % NG) * 128 : (i % NG + 1) * 128],
                                     wrA[j][:, 0:256],
                                     start=(j == 0), stop=(j == NCHUNK - 1))
                pend.append(emit_prep_chain(st, i, qkp, None, p0=True))

            def pair_prep_quanta(st):
                """Pair p>0 prep as PE quanta: per tile 3x (2 matmuls) then
                the vector chain; each tile's transposes are deferred into
                the middle of the NEXT tile so the PE never waits on the
                DVE chain."""
                p = st["p"]
                pend_tr = None
                for i in range(NT):
                    qkp = psPrep.tile([128, 384], dt.float32, tag="qkp")
                    for j3 in range(3):
                        for j in (2 * j3, 2 * j3 + 1):
                            nc.tensor.matmul(qkp[:, 0:256],
                                             xr[j][i // NG][:, (i % NG) * 128 : (i % NG + 1) * 128],
                                             wrB[j][:, (p - 1) * 256 : p * 256],
                                             start=(j == 0), stop=(j == NCHUNK - 1))
                        yield
                    pend_tr_new = emit_prep_chain(st, i, qkp)
                    if pend_tr is not None:
                        pend_tr()
                    pend_tr = pend_tr_new
                    yield
                if pend_tr is not None:
                    pend_tr()

            # ---------------- attention ---------------------------------------
            def att_unit_gen(st, g, hh, off_engine):
                p = st["p"]
                h = 2 * p + hh
                off = 64 * hh
                av = psV.tile([128, G], dt.float32, tag="av")

                def emit_av(kpair, es):
                    for half in range(2):
                        ki = kpair * 2 + half
                        nc.tensor.matmul(
                            av[:],
                            vA[ki // NG][:, ki % NG, h, :],
                            es[:, half * 512 : (half + 1) * 512],
                            start=(ki == 0), stop=(ki == NT - 1),
                        )

                pend = []
                for kpair in range(8):
                    # filler slot FIRST so interleaved prep/proj matmuls run
                    # while the sp-slot / es semaphores settle.
                    yield
                    sp = psS.tile([128, 1024], dt.float32, tag="sp")
                    for half in range(2):
                        ki = kpair * 2 + half
                        nc.tensor.matmul(
                            sp[:, half * 512 : (half + 1) * 512],
                            st["qkT"][ki // NG][off : off + 64, 1,
                                                (ki % NG) * 128 : (ki % NG + 1) * 128],
                            st["qkT"][g][off : off + 64, 0, :],
                            start=True, stop=True,
                        )
                    es = p2e.tile([128, 1024], dt.float32r, name="est", tag="es")
                    nc.scalar.activation(es[:], sp[:], AF.Exp, scale=SCALE)
                    pend.append((kpair, es))
                    if len(pend) > 2:
                        emit_av(*pend.pop(0))
                while pend:
                    emit_av(*pend.pop(0))
                rd = den.tile([64, G], dt.float32, tag="rd")
                nc.vector.reciprocal(rd[:], av[64:128, :])
                nc.vector.tensor_tensor(
                    oT[p][g][off : off + 64, :],
                    av[0:64, :], rd[:], op=ALU.mult,
                )

            # ---------------- projection --------------------------------------
            psProj = None

            def proj_tile_quanta(i):
                if i >= 12 and i % 2 == 0:
                    # tail tiles: the attention score ring is idle by now --
                    # borrow its slots so the last tiles don't serialize on
                    # the single psProj buffer
                    pj_t = psS.tile([128, 1024], dt.float32, name="pjsp", tag="sp")
                    pj = pj_t[:, 0:C]
                else:
                    pj_t = psProj.tile([128, C], dt.float32, name="pj", tag="pj", bufs=1)
                    pj = pj_t[:]
                for pp_ in range(3):
                    st_, spp = (pp_ == 0), (pp_ == 2)
                    sl = oT[pp_][i // NG][:, (i % NG) * 128 : (i % NG + 1) * 128]
                    nc.tensor.matmul(pj[:, 0:512], sl, prW[pp_][:, 0:512], start=st_, stop=spp)
                    nc.tensor.matmul(pj[:, 512:768], sl, prW[pp_][:, 512:768], start=st_, stop=spp)
                    yield
                os_ = outp.tile([128, C], dt.float32, tag="os")
                # split the copy across engines: shortens the pj-ring latency;
                # each half's writeback launches as soon as its copy lands
                if i >= 12:
                    nc.vector.tensor_copy(os_[:, 0:384], pj[:, 0:384])
                    nc.sync.dma_start(out[i * 128 : (i + 1) * 128, 0:384], os_[:, 0:384])
                    nc.scalar.copy(os_[:, 384:768], pj[:, 384:768])
                    nc.sync.dma_start(out[i * 128 : (i + 1) * 128, 384:768], os_[:, 384:768])
                else:
                    nc.vector.tensor_copy(os_[:], pj[:])
                    nc.sync.dma_start(out[i * 128 : (i + 1) * 128, :], os_[:])
                yield

            # ---------------- schedule ----------------------------------------
            def off_eng(unit_idx):
                return "dve" if unit_idx % 2 == 0 else "pool"

            # pair-0 prep up front; the first two attention units' (g=0,
            # hh=0/1) kpairs are gated in as their kT tiles are transposed,
            # interleaved so no single stalled kpair blocks the PE stream.
            cur = new_pair_state(0)
            hgen = [att_unit_gen(cur, 0, 0, off_eng(0)),
                    att_unit_gen(cur, 0, 1, off_eng(1))]
            for hg in hgen:
                next(hg, None)  # advance to the first slot boundary
            # gate kpair j only after the transposes of its kT tiles (2j+1,
            # deferred 2 tiles -> emitted at tile 2j+3) are in the stream
            gates = [(5, 0), (5, 0), (6, 1), (7, 0), (7, 1), (9, 0), (9, 1),
                     (11, 0), (11, 1), (13, 0), (13, 1), (15, 0)]
            pend = []
            p0_v_tile(0)
            p0_v_tile(1)
            for i in range(NT):
                if i + 2 < NT:
                    p0_v_tile(i + 2)
                p0_qk_tile(cur, i, pend)
                while gates and i >= gates[0][0]:
                    next(hgen[gates.pop(0)[1]], None)
            pend.pop(0)()          # tile 14 transposes
            next(hgen[1], None)    # u01 kp5
            pend.pop(0)()          # tile 15 transposes
            next(hgen[1], None)    # u01 kp6
            for hg in hgen:
                for _ in hg:
                    pass

            unit_idx = 2
            proj_state = {"g": 0, "queue": list(range(NT)), "gen": None}
            _SENT = object()

            def pump_proj(n):
                for _ in range(n):
                    if proj_state["gen"] is not None:
                        if next(proj_state["gen"], _SENT) is not _SENT:
                            continue
                        proj_state["gen"] = None
                    if proj_state["queue"] and proj_state["queue"][0] // NG < proj_state["g"]:
                        proj_state["gen"] = proj_tile_quanta(proj_state["queue"].pop(0))
                        next(proj_state["gen"], _SENT)
                    else:
                        return


            filler = None
            for p in range(3):
                if p < 2:
                    nxt = new_pair_state(p + 1)
                    filler = pair_prep_quanta(nxt)
                    n_units = 6 if p == 0 else 8
                    fill_state = {"rate": 4 * NT / (8 * n_units), "acc": 0.0}
                else:
                    nxt = None
                    psPrep_cm.__exit__(None, None, None)
                    psProj_cm = tc.tile_pool(name="psProj", bufs=1, space="PSUM")
                    psProj = psProj_cm.__enter__()
                for g in range(NG):
                    if p == 2:
                        proj_state["g"] = g
                    for hh in range(2):
                        if p == 0 and g == 0:
                            continue
                        gen = att_unit_gen(cur, g, hh, off_eng(unit_idx))
                        unit_idx += 1
                        for _ in gen:
                            if p < 2:
                                # fractional pacing: emit quanta at a steady
                                # rate so prep neither lags nor bursts ahead
                                fill_state["acc"] += fill_state["rate"]
                                while fill_state["acc"] >= 1.0:
                                    fill_state["acc"] -= 1.0
                                    next(filler, _SENT)
                            else:
                                pump_proj(1)
                                proj_state["acc"] = proj_state.get("acc", 0.0) + 0.25
                                if proj_state["acc"] >= 1.0:
                                    proj_state["acc"] -= 1.0
                                    pump_proj(1)
                if p < 2:
                    for _ in filler:
                        pass
                    cur = nxt
            proj_state["g"] = NG
            # drain order: pj-ring tile first (no sp-ring wait), then the
            # sp-borrowing tiles while the last exps retire, pj tile last
            if proj_state["queue"] == [12, 13, 14, 15]:
                proj_state["queue"] = [13, 12, 14, 15]
            while proj_state["queue"] or proj_state["gen"] is not None:
                pump_proj(1)
            psProj_cm.__exit__(None, None, None)
            xw_cm.__exit__(None, None, None)

    nc.compile()
    return nc


_NC = None


def _get_nc():
    global _NC
    if _NC is None:
        _NC = build_program()
    return _NC


def _prep_inputs(x, cos, sin, qkv_w, q_norm_w, k_norm_w, proj_w):
    cos2 = np.asarray(cos, np.float32).reshape(N, D // 2)
    sin2 = np.asarray(sin, np.float32).reshape(N, D // 2)
    cos_full = np.concatenate([cos2, cos2], axis=1)          # [N, 64]
    sin_signed = np.concatenate([-sin2, sin2], axis=1)       # [N, 64]

    def tables(w):
        w = np.asarray(w, np.float32)
        wswap = np.concatenate([w[D // 2 :], w[: D // 2]])
        cw = (8.0 * cos_full * w[None, :]).astype(np.float32)
        sw = (8.0 * sin_signed * wswap[None, :]).astype(np.float32)
        return cw, sw

    cwq_, swq_ = tables(q_norm_w)
    cwk_, swk_ = tables(k_norm_w)

    def shuffle_tab(a):
        # [N, 2D] -> on-chip [partition 128, (t, qk, d)] layout, contiguous
        a = a.reshape(N // 128, 128, 2 * D).transpose(1, 0, 2).reshape(128, -1)
        return np.ascontiguousarray(a.astype(ml_dtypes.bfloat16))

    cqk_ = shuffle_tab(np.stack([cwq_, cwk_], axis=1).reshape(N, 2 * D))
    sqk_ = shuffle_tab(np.stack([swq_, swk_], axis=1).reshape(N, 2 * D))

    in_maps = []
    for c in range(8):
        b, hg = c // 2, c % 2
        h0 = HPC * hg
        rows = np.r_[h0 * D : (h0 + HPC) * D]
        wq = qkv_w[rows]          # [384, C]
        wk = qkv_w[C + rows]
        wv = qkv_w[2 * C + rows]
        # pack as [q0|k0 (256), v (384), q1|k1, q2|k2]
        parts = [wq[0:128], wk[0:128], wv]
        for p in range(1, 3):
            parts.append(wq[p * 128 : (p + 1) * 128])
            parts.append(wk[p * 128 : (p + 1) * 128])
        wqkvT_ = np.ascontiguousarray(
            np.concatenate(parts, 0).T.astype(ml_dtypes.bfloat16))
        projT_ = np.ascontiguousarray(
            proj_w[:, rows].T.astype(ml_dtypes.bfloat16))
        xT_ = np.ascontiguousarray(x[b].T.astype(ml_dtypes.bfloat16))
        in_maps.append({
            "xT": xT_, "wqkvT": wqkvT_, "projT": projT_,
            "cqk": cqk_, "sqk": sqk_,
        })
    return in_maps


def kernel(x, cos, sin, qkv_w, q_norm_w, k_norm_w, proj_w, proj_b, _want_trace=False):
    x = np.asarray(x, np.float32)
    qkv_w = np.asarray(qkv_w, np.float32)
    proj_w = np.asarray(proj_w, np.float32)
    proj_b = np.asarray(proj_b, np.float32)
    in_maps = _prep_inputs(x, cos, sin, qkv_w, q_norm_w, k_norm_w, proj_w)
    nc = _get_nc()
    res = run_bass_kernel_spmd(nc, in_maps, core_ids=list(range(8)), trace=_want_trace)
    out = np.empty((B, N, C), np.float32)
    for b in range(B):
        out[b] = res.results[2 * b]["out"] + res.results[2 * b + 1]["out"] + proj_b[None, :]
    if _want_trace:
        return out, res
    return out
